# revision 93
# baseline (speedup 1.0000x reference)
"""Trainium2 Bass kernel for nn_Attention_60979945668745 (v3).

Multi-head causal attention (B=2, S=2048, D=2048, H=32, hd=64) with
interleaved RoPE, sharded over 8 NeuronCores as DP2 (batch) x TP4 (heads).

Numerics: Q/K projections are 2-series fp8 DoubleRow (w8.x8 + wr8.x8 --
the dropped x-residual term costs ~1.5e-2 end-to-end against the 2e-2
gate); V and the output projection stay 3-series (w8.x8 + wr8.x8 + w8.xr8)
because sharp softmax rows pass V errors through un-averaged.  Scores,
probs and AV run fp16 with fp32 PSUM accumulation.

Structure per core (1 batch, 8 heads, feature slice F=512):
  P1  : Q^T/K^T feature-major with fused RoPE (DVE descale + 6 DVE ops per
        tile, per-head [32 even|32 odd] row blocks); V token-major into
        per-head-slotted V_aug (65th column = ones for softmax sums),
        descaled on ACT.
  attn: S^T tiles [k,q] on PE (fp16, 64-partition contraction), exp on ACT
        (pairs of k-tiles, fp16 out, no max-subtraction), causal-triangle
        fixups via GPSIMD affine_select (SBUF in-place), then token-major
        AV: es slabs stationary -> psum [q, 65] per (head, q-subtile);
        col 64 = softmax sums.  Reciprocal + per-partition scaling
        normalizes during the PSUM->SBUF move.
  P3  : O_tok transposed on PE into O^T, then y = O^T.T @ wo_s per token
        tile; fp16 partials DMA'd out; host sums the 4 TP partials.

Scheduling (the timeline is ACT(exp)-bound during attention and PE-bound
elsewhere, so the emission is software-pipelined at sub-head granularity):
  - att heads run as generators, two heads interleaved pair-of-kt-wise so
    one head's scores hide the other's exp latency;
  - the NEXT chunk's P1 runs as mid-fill quanta dripped between attention
    steps (window qc carries p1(qc+1)); transposes and P3 tiles fill pair
    boundaries and window 3;
  - att3's first head pair slides into window 2 (its accumulator rides
    the then-free p1ps ring);
  - x/weight DMAs ride the SP queue (ACT queue stays clear for exp
    dispatch), interleaved smallest-first at startup because the model
    serializes all transfers on one DMA device;
  - PSUM: p1ps ring 2 banks (P1 + qc3 psA + tail psy), sps 2x2-bank
    score pairs, aps/yps 1 bank each (even/odd head psA, transpose psum,
    P3 psy) -- the placement rules avoid emission-order deadlocks where a
    psum alloc waits on an engine whose queue head waits on later PE work.

TimelineSim: 284.8us/core (baseline 328.3us); rel err 1.475e-2.
"""

import sys

for _p in ("/opt/trn_rl_repo", "/opt/pypackages"):
    if _p not in sys.path:
        sys.path.insert(0, _p)

import numpy as np
import ml_dtypes

import concourse.bacc as bacc
import concourse.mybir as mybir
from concourse.tile import TileContext
from concourse.alu_op_type import AluOpType
from concourse.bass_utils import run_bass_kernel_spmd

F32 = mybir.dt.float32
FP16 = mybir.dt.float16
F8 = mybir.dt.float8e4
AF = mybir.ActivationFunctionType
DRMODE = mybir.MatmulPerfMode.DoubleRow
NPF8 = ml_dtypes.float8_e4m3

DIM = 2048
N_HEADS = 32
HD = 64
BATCH = 2
SEQ = 2048
N_CORES = 8
DP = 2
TP = 4
H_LOC = N_HEADS // TP          # 8 heads per core
F = H_LOC * HD                 # 512 features per core
T = SEQ
N_DT = DIM // 128              # 16 contraction slabs
N_FT = F // 128                # 4 feature tiles
TC = 512                       # q-chunk width
N_TC = T // TC                 # 4 q-chunks
N_KT = T // 128                # 16 k-tiles
SX = 16.0                      # fp8 scale for x
SW = 1024.0                    # fp8 scale for w
DESCALE = 1.0 / (SX * SW)
SO = 32.0                      # fp8 scale for O^T (|O| can reach ~|V|max)
P3DESC = 1.0 / (SO * SW)


def build_nc(block_kind, dbg=False, reps=1, marks=None, upto=99):
    """block_kind[kt][qc] in {'skip','full','tri','mask'}."""
    nc = bacc.Bacc("TRN2", target_bir_lowering=False, debug=False,
                   num_devices=N_CORES)

    x8_d = nc.dram_tensor("x8", [128, N_DT * T], F8, kind="ExternalInput")
    xr8_d = nc.dram_tensor("xr8", [128, N_DT * T], F8, kind="ExternalInput")
    w_d = {}
    for nm in ("wq8", "wqr8", "wk8", "wkr8", "wv8", "wvr8"):
        w_d[nm] = nc.dram_tensor(nm, [128, N_DT * F], F8,
                                 kind="ExternalInput")
    tab_d = nc.dram_tensor("tab", [128, N_FT * 2 * T], FP16,
                           kind="ExternalInput")
    wo8_d = nc.dram_tensor("wo8", [128, N_FT * DIM], F8,
                           kind="ExternalInput")
    wor8_d = nc.dram_tensor("wor8", [128, N_FT * DIM], F8,
                            kind="ExternalInput")
    id_d = nc.dram_tensor("ident", [128, 128], FP16, kind="ExternalInput")
    n_mask = sum(1 for kt in range(N_KT) for qc in range(N_TC)
                 if block_kind[kt][qc] == "mask")
    me_d = nc.dram_tensor("maskexp", [128, max(1, n_mask) * TC], FP16,
                          kind="ExternalInput")
    mask_idx = {}
    mi = 0
    for kt in range(N_KT):
        for qc in range(N_TC):
            if block_kind[kt][qc] == "mask":
                mask_idx[(kt, qc)] = mi
                mi += 1
    y_d = nc.dram_tensor("y", [T, DIM], FP16, kind="ExternalOutput")
    if dbg:
        qh_dump = nc.dram_tensor("qh_dump", [128, N_FT * T], FP16,
                                 kind="ExternalOutput")
        kh_dump = nc.dram_tensor("kh_dump", [128, N_FT * T], FP16,
                                 kind="ExternalOutput")
        v_dump = nc.dram_tensor("v_dump", [128, N_KT * H_LOC * 65], FP16,
                                kind="ExternalOutput")
        ot_dump = nc.dram_tensor("ot_dump", [128, N_FT * T], F8,
                                 kind="ExternalOutput")

    def mark(name):
        if marks is not None:
            marks[name] = int(nc.get_next_instruction_name()[2:])

    with TileContext(nc) as tc_:
        with tc_.tile_pool(name="persist", bufs=1) as persist, \
             tc_.tile_pool(name="w8", bufs=1) as wpool, \
             tc_.tile_pool(name="xs", bufs=2) as xpool, \
             tc_.tile_pool(name="tab", bufs=4) as tabpool, \
             tc_.tile_pool(name="rt", bufs=3) as rtpool, \
             tc_.tile_pool(name="es", bufs=8) as espool, \
             tc_.tile_pool(name="otk", bufs=2) as otkpool, \
             tc_.tile_pool(name="rc", bufs=2) as rcpool, \
             tc_.tile_pool(name="ys", bufs=2) as yspool, \
             tc_.tile_pool(name="p1ps", bufs=2, space="PSUM") as p1ps, \
             tc_.tile_pool(name="sps", bufs=2, space="PSUM") as sps, \
             tc_.tile_pool(name="aps", bufs=1, space="PSUM") as aps, \
             tc_.tile_pool(name="yps", bufs=1, space="PSUM") as yps:

            # ---- persistent tiles ----
            qh = [persist.tile([128, T], FP16, tag=f"qh{ft}",
                               name=f"qh{ft}") for ft in range(N_FT)]
            kh = [persist.tile([128, T], FP16, tag=f"kh{ft}",
                               name=f"kh{ft}") for ft in range(N_FT)]
            v_sb = persist.tile([128, N_KT * H_LOC * 65], FP16, tag="vsb")
            id_sb = persist.tile([128, 128], FP16, tag="id")
            ot8_sb = persist.tile([128, N_FT * T], F8, tag="ot8")
            otr8_sb = persist.tile([128, N_FT * T], F8, tag="otr8")
            wo8_sb = persist.tile([128, N_FT * DIM], F8, tag="wo8")
            wor8_sb = persist.tile([128, N_FT * DIM], F8, tag="wor8")
            me_sb = (persist.tile([128, n_mask * TC], FP16, tag="me")
                     if n_mask else None)

            # ones columns of V_aug (col 64 of each 65-slot)
            ones_view = v_sb[:, :].rearrange("p (s c) -> p s c",
                                             c=65)[:, :, 64]
            nc.gpsimd.memset(ones_view, 1.0)

            # weights (fp8 main + residual), d-major columns.  DMAs are
            # emitted inside the first step, interleaved with the x chunk-0
            # quarters, so the single DMA transfer resource feeds the first
            # P1 tile as early as possible.
            w_sb = {}
            for nm in ("wq8", "wqr8", "wk8", "wkr8", "wv8", "wvr8"):
                w_sb[nm] = wpool.tile([128, N_DT * F], F8, tag=nm, name=nm)

            for _rep in range(reps):
                # streamed x chunks (fp8 main + residual)
                xc8 = [None] * N_TC
                xcr8 = [None] * N_TC

                def load_x(c):
                    # x DMAs ride the SP queue: the scalar (ACT) queue is
                    # kept clear so exp dispatch is never blocked behind a
                    # parked DMA wait.
                    t8 = xpool.tile([128, N_DT * TC], F8, tag="x8")
                    tr8 = xpool.tile([128, N_DT * TC], F8, tag="xr8")
                    cs = slice(c * TC, (c + 1) * TC)
                    iv8 = x8_d[:, :].rearrange("p (d t) -> p d t",
                                               t=T)[:, :, cs]
                    ivr = xr8_d[:, :].rearrange("p (d t) -> p d t",
                                                t=T)[:, :, cs]
                    t8v = t8[:, :].rearrange("p (d t) -> p d t", t=TC)
                    tr8v = tr8[:, :].rearrange("p (d t) -> p d t", t=TC)
                    nc.sync.dma_start(out=t8v, in_=iv8)
                    nc.sync.dma_start(out=tr8v, in_=ivr)
                    xc8[c], xcr8[c] = t8, tr8

                def startup(c=0):
                    """Interleaved weight + x chunk-0 DMAs, smallest-first,
                    so the first P1 tile's operands stream in consumption
                    order through the serialized DMA resource."""
                    t8 = xpool.tile([128, N_DT * TC], F8, tag="x8")
                    tr8 = xpool.tile([128, N_DT * TC], F8, tag="xr8")
                    iv8 = x8_d[:, :].rearrange("p (d t) -> p d t",
                                               t=T)[:, :, 0:TC]
                    ivr = xr8_d[:, :].rearrange("p (d t) -> p d t",
                                                t=T)[:, :, 0:TC]
                    t8v = t8[:, :].rearrange("p (d t) -> p d t", t=TC)
                    tr8v = tr8[:, :].rearrange("p (d t) -> p d t", t=TC)
                    wq8v = w_sb["wq8"][:, :]
                    cuts = [0, 2, 8, N_DT]
                    for a, b in zip(cuts[:-1], cuts[1:]):
                        nc.sync.dma_start(out=wq8v[:, a * F:b * F],
                                          in_=w_d["wq8"][:, a * F:b * F])
                        nc.sync.dma_start(out=t8v[:, a:b, :],
                                          in_=iv8[:, a:b, :])
                    H8 = 8 * F
                    # Q/K are 2-series: xr8 is only consumed by the V
                    # tiles, so it loads after the K weights.
                    nc.sync.dma_start(out=w_sb["wqr8"][:, 0:H8],
                                      in_=w_d["wqr8"][:, 0:H8])
                    nc.sync.dma_start(out=w_sb["wqr8"][:, H8:2 * H8],
                                      in_=w_d["wqr8"][:, H8:2 * H8])
                    for nm in ("wk8", "wkr8"):
                        nc.sync.dma_start(out=w_sb[nm][:, 0:H8],
                                          in_=w_d[nm][:, 0:H8])
                        nc.sync.dma_start(out=w_sb[nm][:, H8:2 * H8],
                                          in_=w_d[nm][:, H8:2 * H8])
                    nc.sync.dma_start(out=tr8v[:, 0:8, :],
                                      in_=ivr[:, 0:8, :])
                    nc.sync.dma_start(out=tr8v[:, 8:N_DT, :],
                                      in_=ivr[:, 8:N_DT, :])
                    for nm in ("wv8", "wvr8"):
                        nc.sync.dma_start(out=w_sb[nm][:, :],
                                          in_=w_d[nm][:, :])
                    xc8[0], xcr8[0] = t8, tr8

                def late_weights():
                    # id/wo8/wor8 are first needed by tp0/p3 in window 1 --
                    # load them after x1 so they don't delay the pipeline
                    nc.sync.dma_start(out=id_sb[:, :], in_=id_d[:, :])
                    if n_mask:
                        nc.sync.dma_start(out=me_sb[:, :], in_=me_d[:, :])
                    nc.sync.dma_start(out=wo8_sb[:, :], in_=wo8_d[:, :])
                    nc.sync.dma_start(out=wor8_sb[:, :], in_=wor8_d[:, :])

                def p1_gen(c):
                    """Q,K (feature-major + RoPE) and V (token-major) for
                    token chunk c.  A generator yielding after each
                    ~8-matmul quantum so attention emission can interleave
                    this PE-dense fill into its exp-bound stream."""
                    mark(f"p1c{c}")
                    x8t, xr8t = xc8[c], xcr8[c]
                    xv8 = x8t[:, :].rearrange("p (d t) -> p d t", t=TC)
                    xvr = xr8t[:, :].rearrange("p (d t) -> p d t", t=TC)
                    qk_order = [("wq8", "wqr8", qh, 0),
                                ("wq8", "wqr8", qh, 1),
                                ("wk8", "wkr8", kh, 0),
                                ("wk8", "wkr8", kh, 1),
                                ("wq8", "wqr8", qh, 2),
                                ("wq8", "wqr8", qh, 3),
                                ("wk8", "wkr8", kh, 2),
                                ("wk8", "wkr8", kh, 3)]
                    tab_tiles = {}
                    for (w8nm, wr8nm, dst, ft) in qk_order:
                        wv8 = w_sb[w8nm][:, :].rearrange(
                            "p (d f) -> p d f", f=F)
                        wvr = w_sb[wr8nm][:, :].rearrange(
                            "p (d f) -> p d f", f=F)
                        if ft in tab_tiles:
                            cs_t = tab_tiles[ft]
                        else:
                            cs_t = tabpool.tile([128, 2 * TC], FP16,
                                                tag="cst", name="cs_t")
                            tin = tab_d[:, :].rearrange(
                                "p (f two t) -> p f two t", f=N_FT,
                                two=2)[:, ft, :, c * TC:(c + 1) * TC]
                            nc.gpsimd.dma_start(
                                out=cs_t[:, :].rearrange(
                                    "p (two t) -> p two t", two=2),
                                in_=tin)
                            tab_tiles[ft] = cs_t
                        fs = slice(ft * 128, (ft + 1) * 128)
                        ps = p1ps.tile([128, TC], F32, tag="p1")
                        # Q/K run 2-series (w8.x8 + wr8.x8): the dropped
                        # x-residual term costs ~1e-2 end-to-end (gate
                        # 2e-2) and saves 1/3 of the Q/K projection PE
                        # time.  V and P3 stay 3-series -- sharp softmax
                        # rows pass V errors through un-averaged.
                        series = ((wv8, xv8), (wvr, xv8))
                        n_mm = 2 * (N_DT // 2)
                        i_mm = 0
                        for wv_, xv_ in series:
                            for p8 in range(N_DT // 2):
                                nc.tensor.matmul(
                                    ps[:, :],
                                    wv_[:, 2 * p8:2 * p8 + 2, fs],
                                    xv_[:, 2 * p8:2 * p8 + 2, :],
                                    start=(i_mm == 0),
                                    stop=(i_mm == n_mm - 1),
                                    perf_mode=DRMODE)
                                i_mm += 1
                            yield
                        # RoPE: stage (descale), rotate (DVE).  The
                        # descale reads PSUM, so it can ride ACT; chunks
                        # 0-2 do (ACT has slack there), chunk 3's land in
                        # the exp-saturated tail so they stay on DVE.
                        t_st = rtpool.tile([128, TC], FP16, tag="tst")
                        if c < N_TC - 1:
                            nc.scalar.activation(t_st[:, :], ps[:, :],
                                                 AF.Copy, scale=DESCALE)
                        else:
                            nc.vector.tensor_scalar_mul(t_st[:, :],
                                                        ps[:, :], DESCALE)
                        t1 = rtpool.tile([128, TC], FP16, tag="t1")
                        nc.vector.tensor_mul(t1[:, :], t_st[:, :],
                                             cs_t[:, 0:TC])
                        cs = slice(c * TC, (c + 1) * TC)
                        st_tab = cs_t[:, TC:2 * TC]
                        # st table rows are pre-swapped host-side so
                        # both DVE inputs share a base partition
                        # (walrus NCC_IBIR297).
                        for hb in (0, 64):
                            nc.vector.tensor_mul(
                                dst[ft][hb:hb + 32, cs],
                                t_st[hb + 32:hb + 64, :],
                                st_tab[hb + 32:hb + 64, :])
                            nc.vector.tensor_mul(
                                dst[ft][hb + 32:hb + 64, cs],
                                t_st[hb:hb + 32, :],
                                st_tab[hb:hb + 32, :])
                        nc.vector.tensor_add(dst[ft][:, cs],
                                             dst[ft][:, cs], t1[:, :])
                        yield
                    # V token-major
                    wv8 = w_sb["wv8"][:, :].rearrange("p (d f) -> p d f",
                                                      f=F)
                    wvr = w_sb["wvr8"][:, :].rearrange("p (d f) -> p d f",
                                                      f=F)
                    for tt in range(TC // 128):
                        kt = c * (TC // 128) + tt
                        tsl = slice(tt * 128, (tt + 1) * 128)
                        psv = p1ps.tile([128, F], F32, tag="p1")
                        series = ((xv8, wv8), (xv8, wvr), (xvr, wv8))
                        n_mm = 3 * (N_DT // 2)
                        i_mm = 0
                        for xv_, wv_ in series:
                            for p8 in range(N_DT // 2):
                                nc.tensor.matmul(
                                    psv[:, :],
                                    xv_[:, 2 * p8:2 * p8 + 2, tsl],
                                    wv_[:, 2 * p8:2 * p8 + 2, :],
                                    start=(i_mm == 0),
                                    stop=(i_mm == n_mm - 1),
                                    perf_mode=DRMODE)
                                i_mm += 1
                            yield
                        base = kt * H_LOC * 65
                        vout = v_sb[:, base:base + H_LOC * 65].rearrange(
                            "p (h c) -> p h c", c=65)[:, :, 0:64]
                        vin = psv[:, :].rearrange("p (h c) -> p h c", c=64)
                        # ACT can read PSUM (GPSIMD cannot); keeps DVE free
                        # for the RoPE stream.
                        nc.scalar.activation(vout, vin, AF.Copy,
                                             scale=DESCALE)
                        yield

                def att_ctx(qc, reg=None):
                    kinds = [block_kind[kt][qc] for kt in range(N_KT)]
                    live = [kt for kt in range(N_KT) if kinds[kt] != "skip"]
                    if not live:
                        return None
                    otok = otkpool.tile([128, N_TC * TC], FP16, tag="otok")
                    if reg is not None:
                        reg[qc] = otok
                    offs = {}
                    for kt in live:
                        if kinds[kt] == "tri":
                            offs[kt] = max(0, kt * 128 - qc * TC)
                        else:
                            offs[kt] = 0
                    R = TC // 128
                    lv = {qt: [kt for kt in live if offs[kt] <= qt * 128]
                          for qt in range(R)}
                    pairs = [live[i:i + 2] for i in range(0, len(live), 2)]
                    return dict(qc=qc, kinds=kinds, offs=offs, lv=lv,
                                pairs=pairs, otok=otok, R=R)

                def att_head_gen(ctx, h):
                    """One head\'s attention as a generator: step j emits
                    AV(pair j-1) + S/exp(pair j).  Two heads driven
                    alternately + PE fill quanta between steps hide the
                    exp latency."""
                    qc = ctx["qc"]
                    mark(f"a{qc}h{h}")
                    kinds, offs, lv = ctx["kinds"], ctx["offs"], ctx["lv"]
                    otok, R = ctx["otok"], ctx["R"]
                    ft_h = h // 2
                    po = (h % 2) * 64
                    qs = qh[ft_h][po:po + 64, :]
                    ks = kh[ft_h][po:po + 64, :]
                    # accumulator: even heads on aps, odd heads on yps; the
                    # last chunk borrows the dead projection ring so yps and
                    # aps stay free for the P3/tp fillers.
                    if qc == N_TC - 1:
                        psA_t = p1ps.tile([128, TC], F32, tag="p1",
                                          name="psA_t")
                        psA = psA_t[:, 0:R * 65]
                    elif h % 2 == 0:
                        psA = aps.tile([128, R * 65], F32, tag="av")
                    else:
                        psA_t = yps.tile([128, TC], F32, tag="y",
                                         name="psA_t")
                        psA = psA_t[:, 0:R * 65]
                    # start=True ONLY on the first AV matmul of the head:
                    # it zeroes the whole 2KB zero-region (all qt slots);
                    # later slots first-touch as overwrite-pending (= add
                    # onto zero), then pure-accumulate.  A start on any
                    # later matmul would wipe sibling slots.
                    first_mm = [True]

                    def S(pair):
                        ps = sps.tile([128, 2 * TC], F32, tag="s2")
                        es = espool.tile([128, 2 * TC], FP16, tag="es")
                        for i, kt in enumerate(pair):
                            off = offs[kt]
                            nc.tensor.matmul(
                                ps[:, i * TC + off:(i + 1) * TC],
                                ks[:, kt * 128:(kt + 1) * 128],
                                qs[:, qc * TC + off:(qc + 1) * TC],
                                start=True, stop=True)
                        lo = offs[pair[0]]
                        hi = len(pair) * TC
                        nc.scalar.activation(es[:, lo:hi], ps[:, lo:hi],
                                             AF.Exp, scale=0.125)
                        for i, kt in enumerate(pair):
                            off = offs[kt]
                            if kinds[kt] == "tri":
                                # causal triangle fixup on GPSIMD: zero
                                # es[p, j] where j < p (j relative to the
                                # block\'s diagonal at column off)
                                esl = es[:, i * TC + off:i * TC + off + 128]
                                nc.gpsimd.affine_select(
                                    esl, esl, pattern=[[1, 128]],
                                    compare_op=AluOpType.is_ge,
                                    fill=0.0, base=0,
                                    channel_multiplier=-1)
                            elif kinds[kt] == "mask":
                                ms = mask_idx[(kt, qc)]
                                nc.gpsimd.tensor_mul(
                                    es[:, i * TC:(i + 1) * TC],
                                    es[:, i * TC:(i + 1) * TC],
                                    me_sb[:, ms * TC:(ms + 1) * TC])
                        return es

                    def AV(pair, es):
                        # token-major AV (ones column -> softmax sums)
                        for i, kt in enumerate(pair):
                            for qt in range(R):
                                if kt not in lv[qt]:
                                    continue
                                nc.tensor.matmul(
                                    psA[:, qt * 65:qt * 65 + 65],
                                    es[:, i * TC + qt * 128:
                                       i * TC + (qt + 1) * 128],
                                    v_sb[:, kt * H_LOC * 65 + h * 65:
                                         kt * H_LOC * 65 + h * 65 + 65],
                                    start=first_mm[0],
                                    stop=(kt == lv[qt][-1]),
                                    skip_group_check=True)
                                first_mm[0] = False

                    pairs = ctx["pairs"]
                    prev = pairs[0]
                    es_prev = S(prev)
                    yield
                    for pair in pairs[1:]:
                        AV(prev, es_prev)
                        prev, es_prev = pair, S(pair)
                        yield
                    AV(prev, es_prev)
                    rcp = rcpool.tile([128, R], F32, tag="rcp")
                    sums_v = psA[:, :].rearrange("p (r c) -> p r c",
                                                 c=65)[:, :, 64]
                    nc.vector.reciprocal_approx_fast(rcp[:, :], sums_v)
                    in0 = psA[:, :].rearrange("p (r c) -> p r c",
                                              c=65)[:, :, 0:64]
                    in1 = rcp[:, :].unsqueeze(2).broadcast_to([128, R, 64])
                    out_v = otok[:, :].rearrange(
                        "p (r f) -> p r f", f=TC)[:, :, h * 64:(h + 1) * 64]
                    nc.vector.tensor_mul(out_v, in0, in1)

                def tp_gen(qc, otok, fks=None, qts=None):
                    # transpose O_tok -> O^T (feature-major)
                    mark(f"tp{qc}")
                    Rl = TC // 128
                    for qt in (range(Rl) if qts is None else qts):
                        for fk in (range(N_FT) if fks is None else fks):
                            pst = aps.tile([128, Rl * 65], F32, tag="av")
                            tpv = pst[:, 0:64].bitcast(FP16)
                            nc.tensor.transpose(
                                tpv,
                                otok[:, qt * TC + fk * 128:
                                     qt * TC + (fk + 1) * 128],
                                id_sb[:, :])
                            tglob = qc * Rl + qt
                            osl = slice(fk * T + tglob * 128,
                                        fk * T + (tglob + 1) * 128)
                            nc.vector.tensor_scalar_mul(
                                ot8_sb[:, osl], tpv, SO)
                            nc.vector.scalar_tensor_tensor(
                                otr8_sb[:, osl], tpv, SO,
                                ot8_sb[:, osl], AluOpType.mult,
                                AluOpType.subtract)
                        yield

                def p3_gen(tt, mode="w12", last=False):
                    """P3 y tile for token tile tt, yielding per dc so the
                    psy ring waits hide under interleaved attention steps.
                    mode picks which psum banks psy may ride:
                      w12 : yps / p1ps alternate (windows 1-2)
                      w3  : yps / aps alternate (window 3: p1ps = psA)
                      tail: yps / sps halves / p1ps rotation (post-att)
                    """
                    mark(f"p3t{tt}")
                    ysr = yspool.tile([128, DIM], FP16, tag="ysr")
                    sps_t = [None]
                    for dc in range(DIM // TC):
                        if dc % 2 == 0 and mode != "tail":
                            psy = yps.tile([128, TC], F32, tag="y")
                        elif mode == "w12":
                            psy = p1ps.tile([128, TC], F32, tag="p1",
                                            name="psyB")
                        elif mode == "w3":
                            psy = aps.tile([128, TC], F32, tag="av",
                                           name="psyA")
                        elif mode == "tail":
                            if dc == 0:
                                psy = yps.tile([128, TC], F32, tag="y")
                            elif dc == 2:
                                psy = p1ps.tile([128, TC], F32, tag="p1",
                                                name="psy3")
                            else:
                                if sps_t[0] is None:
                                    sps_t[0] = sps.tile([128, 2 * TC], F32,
                                                        tag="s2",
                                                        name="psyS")
                                psy = sps_t[0][:, (dc // 2) * TC:
                                               (dc // 2) * TC + TC]
                        i_mm = 0
                        for (lt, rt) in ((ot8_sb, wo8_sb),
                                         (otr8_sb, wo8_sb),
                                         (ot8_sb, wor8_sb)):
                            ltv = lt[:, :].rearrange("p (f t) -> p f t",
                                                     t=T)
                            rtv = rt[:, :].rearrange("p (f d) -> p f d",
                                                     d=DIM)
                            for fkp in range(N_FT // 2):
                                nc.tensor.matmul(
                                    psy[:, :],
                                    ltv[:, 2 * fkp:2 * fkp + 2,
                                        tt * 128:(tt + 1) * 128],
                                    rtv[:, 2 * fkp:2 * fkp + 2,
                                        dc * TC:(dc + 1) * TC],
                                    start=(i_mm == 0), stop=(i_mm == 5),
                                    perf_mode=DRMODE)
                                i_mm += 1
                        if dc % 2 == 1 and mode != "w3" and not last:
                            nc.scalar.activation(
                                ysr[:, dc * TC:(dc + 1) * TC],
                                psy[:, :], AF.Copy, scale=P3DESC)
                        else:
                            nc.vector.tensor_scalar_mul(
                                ysr[:, dc * TC:(dc + 1) * TC], psy[:, :],
                                P3DESC)
                        if mode == "tail" and last:
                            # last tile: store per dc so the final DMA
                            # chain starts as early as possible
                            nc.sync.dma_start(
                                out=y_d[tt * 128:(tt + 1) * 128,
                                        dc * TC:(dc + 1) * TC],
                                in_=ysr[:, dc * TC:(dc + 1) * TC])
                        elif mode == "tail" and dc % 2 == 1:
                            nc.sync.dma_start(
                                out=y_d[tt * 128:(tt + 1) * 128,
                                        (dc - 1) * TC:(dc + 1) * TC],
                                in_=ysr[:, (dc - 1) * TC:(dc + 1) * TC])
                        yield
                    if mode != "tail":
                        nc.sync.dma_start(
                            out=y_d[tt * 128:(tt + 1) * 128, :],
                            in_=ysr[:, :])

                def drive(pair_gens, mid=(), per_step=1.0, boundary=None,
                          drain_at=None):
                    """Drive head-pair generator groups; advance ~per_step
                    mid-fill quanta per attention step; drain boundary
                    generators after each head pair.  drain_at forces the
                    mid fill to finish before that pair index (used when a
                    slid-in next-chunk pair consumes the fill's output)."""
                    mid = list(mid)
                    quota = [0.0]

                    def adv(n):
                        quota[0] += n
                        while quota[0] >= 1.0 and mid:
                            try:
                                next(mid[0])
                            except StopIteration:
                                mid.pop(0)
                                continue
                            quota[0] -= 1.0

                    for pi, gens in enumerate(pair_gens):
                        gens = list(gens)
                        while gens:
                            for g_ in list(gens):
                                try:
                                    next(g_)
                                except StopIteration:
                                    gens.remove(g_)
                                    continue
                                adv(per_step)
                        if boundary and pi in boundary:
                            for g_ in boundary[pi]:
                                for _ in g_:
                                    pass
                    for g_ in mid:
                        for _ in g_:
                            pass

                def hp(ctx):
                    return [[att_head_gen(ctx, h), att_head_gen(ctx, h + 1)]
                            for h in range(0, H_LOC, 2)]

                # ---- interleaved emission (software pipeline) ----
                R = TC // 128
                st_ot = {}
                startup()
                load_x(1)
                late_weights()
                for _ in p1_gen(0):
                    pass
                load_x(2)
                # window 0: att0 (tiny) carries p1c1 as fill
                drive(hp(att_ctx(0, reg=st_ot)), mid=[p1_gen(1)],
                      per_step=3.0)
                load_x(3)
                # window 1: att1 + p1c2 fill; tp0/p3(0..3) at boundaries
                drive(hp(att_ctx(1, reg=st_ot)), mid=[p1_gen(2)],
                      per_step=1.35,
                      boundary={0: [tp_gen(0, st_ot[0])],
                                1: [p3_gen(0, mode="w3")],
                                2: [p3_gen(1, mode="w3")],
                                3: [p3_gen(2, mode="w3"),
                                    p3_gen(3, mode="w3")]})
                # window 2: att2 + p1c3 fill, then att3's first head pair
                # as a 5th top-level pair (its psA rides the p1ps ring,
                # free once the p1c3 mid-fill has drained); p3(4..5) at
                # boundaries on yps/aps ("w3" mode).
                ctx2 = att_ctx(2, reg=st_ot)
                ctx3 = att_ctx(3, reg=st_ot)
                drive(hp(ctx2) +
                      [[att_head_gen(ctx3, 0), att_head_gen(ctx3, 1)]],
                      mid=[p1_gen(3)],
                      per_step=1.0,
                      boundary={0: [tp_gen(1, st_ot[1])],
                                1: [p3_gen(4, mode="w3")],
                                2: [p3_gen(5, mode="w3")]})  # all-DVE copies
                # window 3: att3 heads 2-7; tp3.fk0 valid already (heads
                # 0-1 done), further fk-slices as pairs complete
                drive([[att_head_gen(ctx3, h), att_head_gen(ctx3, h + 1)]
                       for h in (2, 4, 6)],
                      mid=[p3_gen(6, mode="w3"),
                           p3_gen(7, mode="w3"),
                           tp_gen(2, st_ot[2]),
                           tp_gen(3, st_ot[3], fks=[0])] +
                          [p3_gen(t, mode="w3") for t in range(8, 12)],
                      per_step=0.65,
                      boundary={0: [tp_gen(3, st_ot[3], fks=[1])],
                                1: [tp_gen(3, st_ot[3], fks=[2])]})
                for _ in tp_gen(3, st_ot[3], fks=[3]):
                    pass
                for t in range(3 * R, 4 * R):
                    for _ in p3_gen(t, mode="tail", last=(t == 4 * R - 1)):
                        pass

                if dbg:
                    for ft in range(N_FT):
                        nc.sync.dma_start(
                            out=qh_dump[:, ft * T:(ft + 1) * T],
                            in_=qh[ft][:, :])
                        nc.sync.dma_start(
                            out=kh_dump[:, ft * T:(ft + 1) * T],
                            in_=kh[ft][:, :])
                        nc.sync.dma_start(
                            out=ot_dump[:, ft * T:(ft + 1) * T],
                            in_=ot8_sb[:, ft * T:(ft + 1) * T])
                    nc.sync.dma_start(out=v_dump[:, :], in_=v_sb[:, :])

    nc.compile()
    return nc


# ---------------- host-side preparation ----------------

def _perm():
    p = np.empty(HD, np.int64)
    p[:32] = np.arange(0, HD, 2)
    p[32:] = np.arange(1, HD, 2)
    return p


def _q8_pair(a, scale):
    """Same-grid fp8 main + residual (both at `scale`)."""
    v = np.asarray(a, np.float32) * scale
    a8 = v.astype(NPF8)
    r = v - a8.astype(np.float32)
    r8 = r.astype(NPF8)
    return a8, r8


def _pack_dmajor(mat, inner):
    """[D, inner] -> [128, N_DT*inner], col = d*inner + j."""
    d = mat.shape[0]
    return np.ascontiguousarray(
        mat.reshape(d // 128, 128, inner).transpose(1, 0, 2).reshape(
            128, (d // 128) * inner))


def _rope_tab(start_pos, heads):
    """[128, N_FT*2*T] fp16: per ft [ct | st]; rows per head block:
    ct = cos(theta_g), st = [-sin; +sin] in 32-row halves."""
    base = 1.0 / (10000.0 ** (np.arange(0, HD, 2, dtype=np.float64) / HD))
    pos = start_pos + np.arange(T, dtype=np.float64)
    tab = np.empty((128, N_FT * 2 * T), np.float32)
    for ft in range(N_FT):
        for hb in range(2):
            g = heads[2 * ft + hb]
            th = base[g] * pos
            c = np.cos(th).astype(np.float32)
            s = np.sin(th).astype(np.float32)
            r0 = hb * 64
            tab[r0:r0 + 64, ft * 2 * T:ft * 2 * T + T] = c[None, :]
            # st rows pre-swapped: top half multiplies the odd block
            # (writes even rows with -sin read from odd partitions), so
            # +sin sits at rows 0:32 and -sin at rows 32:64.
            tab[r0:r0 + 32, ft * 2 * T + T:ft * 2 * T + 2 * T] = s[None, :]
            tab[r0 + 32:r0 + 64,
                ft * 2 * T + T:ft * 2 * T + 2 * T] = -s[None, :]
    return tab.astype(np.float16)


def _classify_mask(mask):
    n_kt, n_tc = N_KT, N_TC
    tri = (np.arange(TC)[None, :] >= np.arange(128)[:, None])
    kinds = [[None] * n_tc for _ in range(n_kt)]
    for kt in range(n_kt):
        for qc in range(n_tc):
            blk = mask[qc * TC:(qc + 1) * TC, kt * 128:(kt + 1) * 128].T
            if np.all(blk <= -1e8):
                kinds[kt][qc] = "skip"
            elif np.all(blk == 0.0):
                kinds[kt][qc] = "full"
            else:
                off = kt * 128 - qc * TC
                is_tri = False
                if 0 <= off <= TC - 128:
                    ref = np.full((128, TC), -1e9, np.float32)
                    ref[:, off:] = np.where(tri[:, :TC - off], 0.0, -1e9)
                    is_tri = bool(np.array_equal(blk, ref))
                kinds[kt][qc] = "tri" if is_tri else "mask"
    slabs = []
    for kt in range(n_kt):
        for qc in range(n_tc):
            if kinds[kt][qc] == "mask":
                blk = mask[qc * TC:(qc + 1) * TC,
                           kt * 128:(kt + 1) * 128].T
                slabs.append(np.exp(blk.astype(np.float64)
                                    ).astype(np.float16))
    me = (np.concatenate(slabs, axis=1) if slabs
          else np.zeros((128, TC), np.float16))
    return kinds, me


def prepare_in_maps(x, wq, wk, wv, wo, mask, start_pos):
    x = np.asarray(x, np.float32)
    wq = np.asarray(wq, np.float32)
    wk = np.asarray(wk, np.float32)
    wv = np.asarray(wv, np.float32)
    wo = np.asarray(wo, np.float32)
    mask2d = np.asarray(mask, np.float32).reshape(mask.shape[-2],
                                                  mask.shape[-1])
    sp = int(np.asarray(start_pos))
    kinds, me = _classify_mask(mask2d)

    ident = np.eye(128, dtype=np.float16)
    perm = _perm()

    in_maps = []
    for core in range(N_CORES):
        b = core // TP
        tp = core % TP
        heads = np.arange(tp * H_LOC, (tp + 1) * H_LOC)
        rows = (heads[:, None] * HD + perm[None, :]).reshape(-1)
        rows_plain = (heads[:, None] * HD
                      + np.arange(HD)[None, :]).reshape(-1)

        xT = np.ascontiguousarray(x[b].T)               # [D, T]
        x8, xr8 = _q8_pair(xT, SX)
        x8 = _pack_dmajor(x8, T)
        xr8 = _pack_dmajor(xr8, T)

        def wpack(w, r):
            wT = np.ascontiguousarray(w[r, :].T)        # [D, F]
            a8, r8 = _q8_pair(wT, SW)
            return _pack_dmajor(a8, F), _pack_dmajor(r8, F)

        wq8, wqr8 = wpack(wq, rows)
        wk8, wkr8 = wpack(wk, rows)
        wv8, wvr8 = wpack(wv, rows_plain)
        wo_sT = np.ascontiguousarray(wo[:, rows_plain].T)  # [F, D]
        wo8, wor8 = _q8_pair(wo_sT, SW)
        wo8 = _pack_dmajor(wo8, DIM)
        wor8 = _pack_dmajor(wor8, DIM)

        in_maps.append({
            "x8": x8, "xr8": xr8,
            "wq8": wq8, "wqr8": wqr8, "wk8": wk8, "wkr8": wkr8,
            "wv8": wv8, "wvr8": wvr8,
            "tab": _rope_tab(sp, heads),
            "wo8": wo8, "wor8": wor8,
            "ident": ident, "maskexp": me,
        })
    return in_maps, kinds


_CACHE = {}


def get_nc(kinds):
    key = tuple(tuple(r) for r in kinds)
    if key not in _CACHE:
        _CACHE[key] = build_nc(kinds)
    return _CACHE[key]


def kernel(x, wq, wk, wv, wo, mask, start_pos):
    in_maps, kinds = prepare_in_maps(x, wq, wk, wv, wo, mask, start_pos)
    nc = get_nc(kinds)
    out = run_bass_kernel_spmd(nc, in_maps, core_ids=list(range(N_CORES)))
    y = np.zeros((BATCH, SEQ, DIM), np.float32)
    for core in range(N_CORES):
        y[core // TP] += out.results[core]["y"].astype(np.float32)
    return y



# revision 108
# speedup vs baseline: 1.0238x; 1.0238x over previous
"""Trainium2 Bass kernel for nn_Attention_60979945668745 (v3).

Multi-head causal attention (B=2, S=2048, D=2048, H=32, hd=64) with
interleaved RoPE, sharded over 8 NeuronCores as DP2 (batch) x TP4 (heads).

Numerics: Q/K projections are 2-series fp8 DoubleRow (w8.x8 + wr8.x8 --
the dropped x-residual term costs ~1.5e-2 end-to-end against the 2e-2
gate); V and the output projection stay 3-series (w8.x8 + wr8.x8 + w8.xr8)
because sharp softmax rows pass V errors through un-averaged.  Scores,
probs and AV run fp16 with fp32 PSUM accumulation.

Structure per core (1 batch, 8 heads, feature slice F=512):
  P1  : Q^T/K^T feature-major with fused RoPE (DVE descale + 6 DVE ops per
        tile, per-head [32 even|32 odd] row blocks); V token-major into
        per-head-slotted V_aug (65th column = ones for softmax sums),
        descaled on ACT.
  attn: S^T tiles [k,q] on PE (fp16, 64-partition contraction), exp on ACT
        (pairs of k-tiles, fp16 out, no max-subtraction), causal-triangle
        fixups via GPSIMD affine_select (SBUF in-place), then token-major
        AV: es slabs stationary -> psum [q, 65] per (head, q-subtile);
        col 64 = softmax sums.  Reciprocal + per-partition scaling
        normalizes during the PSUM->SBUF move.
  P3  : O_tok transposed on PE into O^T, then y = O^T.T @ wo_s per token
        tile; fp16 partials DMA'd out; host sums the 4 TP partials.

Scheduling (the timeline is ACT(exp)-bound during attention and PE-bound
elsewhere, so the emission is software-pipelined at sub-head granularity):
  - att heads run as generators, two heads interleaved pair-of-kt-wise so
    one head's scores hide the other's exp latency;
  - the NEXT chunk's P1 runs as mid-fill quanta dripped between attention
    steps (window qc carries p1(qc+1)); transposes and P3 tiles fill pair
    boundaries and window 3;
  - att3's first head pair slides into window 2 (its accumulator rides
    the then-free p1ps ring);
  - x/weight DMAs ride the SP queue (ACT queue stays clear for exp
    dispatch), interleaved smallest-first at startup because the model
    serializes all transfers on one DMA device;
  - PSUM: p1ps ring 2 banks (P1 + qc3 psA + tail psy), sps 2x2-bank
    score pairs, aps/yps 1 bank each (even/odd head psA, transpose psum,
    P3 psy) -- the placement rules avoid emission-order deadlocks where a
    psum alloc waits on an engine whose queue head waits on later PE work.

TimelineSim: 272.2us/core (baseline 328.3us); rel err 1.475e-2.
"""

import sys

for _p in ("/opt/trn_rl_repo", "/opt/pypackages"):
    if _p not in sys.path:
        sys.path.insert(0, _p)

import numpy as np
import ml_dtypes

import concourse.bacc as bacc
import concourse.mybir as mybir
from concourse.tile import TileContext
from concourse.alu_op_type import AluOpType
from concourse.bass_utils import run_bass_kernel_spmd

F32 = mybir.dt.float32
FP16 = mybir.dt.float16
F8 = mybir.dt.float8e4
AF = mybir.ActivationFunctionType
DRMODE = mybir.MatmulPerfMode.DoubleRow
NPF8 = ml_dtypes.float8_e4m3

DIM = 2048
N_HEADS = 32
HD = 64
BATCH = 2
SEQ = 2048
N_CORES = 8
DP = 2
TP = 4
H_LOC = N_HEADS // TP          # 8 heads per core
F = H_LOC * HD                 # 512 features per core
T = SEQ
N_DT = DIM // 128              # 16 contraction slabs
N_FT = F // 128                # 4 feature tiles
TC = 512                       # q-chunk width
N_TC = T // TC                 # 4 q-chunks
N_KT = T // 128                # 16 k-tiles
SX = 16.0                      # fp8 scale for x
SW = 1024.0                    # fp8 scale for w
DESCALE = 1.0 / (SX * SW)
SO = 32.0                      # fp8 scale for O^T (|O| can reach ~|V|max)
P3DESC = 1.0 / (SO * SW)


def build_nc(block_kind, dbg=False, reps=1, marks=None, upto=99):
    """block_kind[kt][qc] in {'skip','full','tri','mask'}."""
    nc = bacc.Bacc("TRN2", target_bir_lowering=False, debug=False,
                   num_devices=N_CORES)

    x8_d = nc.dram_tensor("x8", [128, N_DT * T], F8, kind="ExternalInput")
    xr8_d = nc.dram_tensor("xr8", [128, N_DT * T], F8, kind="ExternalInput")
    w_d = {}
    for nm in ("wq8", "wqr8", "wk8", "wkr8", "wv8", "wvr8"):
        w_d[nm] = nc.dram_tensor(nm, [128, N_DT * F], F8,
                                 kind="ExternalInput")
    tab_d = nc.dram_tensor("tab", [128, N_FT * 2 * T], FP16,
                           kind="ExternalInput")
    wo8_d = nc.dram_tensor("wo8", [128, N_FT * DIM], F8,
                           kind="ExternalInput")
    wor8_d = nc.dram_tensor("wor8", [128, N_FT * DIM], F8,
                            kind="ExternalInput")
    id_d = nc.dram_tensor("ident", [128, 128], FP16, kind="ExternalInput")
    n_mask = sum(1 for kt in range(N_KT) for qc in range(N_TC)
                 if block_kind[kt][qc] == "mask")
    me_d = nc.dram_tensor("maskexp", [128, max(1, n_mask) * TC], FP16,
                          kind="ExternalInput")
    mask_idx = {}
    mi = 0
    for kt in range(N_KT):
        for qc in range(N_TC):
            if block_kind[kt][qc] == "mask":
                mask_idx[(kt, qc)] = mi
                mi += 1
    y_d = nc.dram_tensor("y", [T, DIM], FP16, kind="ExternalOutput")
    if dbg:
        qh_dump = nc.dram_tensor("qh_dump", [128, N_FT * T], FP16,
                                 kind="ExternalOutput")
        kh_dump = nc.dram_tensor("kh_dump", [128, N_FT * T], FP16,
                                 kind="ExternalOutput")
        v_dump = nc.dram_tensor("v_dump", [128, N_KT * H_LOC * 65], FP16,
                                kind="ExternalOutput")
        ot_dump = nc.dram_tensor("ot_dump", [128, N_FT * T], F8,
                                 kind="ExternalOutput")

    def mark(name):
        if marks is not None:
            marks[name] = int(nc.get_next_instruction_name()[2:])

    with TileContext(nc) as tc_:
        with tc_.tile_pool(name="persist", bufs=1) as persist, \
             tc_.tile_pool(name="w8", bufs=1) as wpool, \
             tc_.tile_pool(name="xs", bufs=2) as xpool, \
             tc_.tile_pool(name="tab", bufs=4) as tabpool, \
             tc_.tile_pool(name="rt", bufs=3) as rtpool, \
             tc_.tile_pool(name="es", bufs=8) as espool, \
             tc_.tile_pool(name="otk", bufs=2) as otkpool, \
             tc_.tile_pool(name="rc", bufs=2) as rcpool, \
             tc_.tile_pool(name="ys", bufs=2) as yspool, \
             tc_.tile_pool(name="p1ps", bufs=2, space="PSUM") as p1ps, \
             tc_.tile_pool(name="sps", bufs=2, space="PSUM") as sps, \
             tc_.tile_pool(name="aps", bufs=1, space="PSUM") as aps, \
             tc_.tile_pool(name="yps", bufs=1, space="PSUM") as yps:

            # ---- persistent tiles ----
            qh = [persist.tile([128, T], FP16, tag=f"qh{ft}",
                               name=f"qh{ft}") for ft in range(N_FT)]
            kh = [persist.tile([128, T], FP16, tag=f"kh{ft}",
                               name=f"kh{ft}") for ft in range(N_FT)]
            v_sb = persist.tile([128, N_KT * H_LOC * 65], FP16, tag="vsb")
            id_sb = persist.tile([128, 128], FP16, tag="id")
            ot8_sb = persist.tile([128, N_FT * T], F8, tag="ot8")
            otr8_sb = persist.tile([128, N_FT * T], F8, tag="otr8")
            wo8_sb = persist.tile([128, N_FT * DIM], F8, tag="wo8")
            wor8_sb = persist.tile([128, N_FT * DIM], F8, tag="wor8")
            me_sb = (persist.tile([128, n_mask * TC], FP16, tag="me")
                     if n_mask else None)

            # ones columns of V_aug (col 64 of each 65-slot)
            ones_view = v_sb[:, :].rearrange("p (s c) -> p s c",
                                             c=65)[:, :, 64]
            nc.gpsimd.memset(ones_view, 1.0)

            # weights (fp8 main + residual), d-major columns.  DMAs are
            # emitted inside the first step, interleaved with the x chunk-0
            # quarters, so the single DMA transfer resource feeds the first
            # P1 tile as early as possible.
            w_sb = {}
            for nm in ("wq8", "wqr8", "wk8", "wkr8", "wv8", "wvr8"):
                w_sb[nm] = wpool.tile([128, N_DT * F], F8, tag=nm, name=nm)

            for _rep in range(reps):
                # streamed x chunks (fp8 main + residual)
                xc8 = [None] * N_TC
                xcr8 = [None] * N_TC

                def load_x(c):
                    # x DMAs ride the SP queue: the scalar (ACT) queue is
                    # kept clear so exp dispatch is never blocked behind a
                    # parked DMA wait.
                    t8 = xpool.tile([128, N_DT * TC], F8, tag="x8")
                    tr8 = xpool.tile([128, N_DT * TC], F8, tag="xr8")
                    cs = slice(c * TC, (c + 1) * TC)
                    iv8 = x8_d[:, :].rearrange("p (d t) -> p d t",
                                               t=T)[:, :, cs]
                    ivr = xr8_d[:, :].rearrange("p (d t) -> p d t",
                                                t=T)[:, :, cs]
                    t8v = t8[:, :].rearrange("p (d t) -> p d t", t=TC)
                    tr8v = tr8[:, :].rearrange("p (d t) -> p d t", t=TC)
                    nc.sync.dma_start(out=t8v, in_=iv8)
                    nc.sync.dma_start(out=tr8v, in_=ivr)
                    xc8[c], xcr8[c] = t8, tr8

                def startup(c=0):
                    """Interleaved weight + x chunk-0 DMAs, smallest-first,
                    so the first P1 tile's operands stream in consumption
                    order through the serialized DMA resource."""
                    t8 = xpool.tile([128, N_DT * TC], F8, tag="x8")
                    tr8 = xpool.tile([128, N_DT * TC], F8, tag="xr8")
                    iv8 = x8_d[:, :].rearrange("p (d t) -> p d t",
                                               t=T)[:, :, 0:TC]
                    ivr = xr8_d[:, :].rearrange("p (d t) -> p d t",
                                                t=T)[:, :, 0:TC]
                    t8v = t8[:, :].rearrange("p (d t) -> p d t", t=TC)
                    tr8v = tr8[:, :].rearrange("p (d t) -> p d t", t=TC)
                    wq8v = w_sb["wq8"][:, :]
                    cuts = [0, 2, 8, N_DT]
                    for a, b in zip(cuts[:-1], cuts[1:]):
                        nc.sync.dma_start(out=wq8v[:, a * F:b * F],
                                          in_=w_d["wq8"][:, a * F:b * F])
                        nc.sync.dma_start(out=t8v[:, a:b, :],
                                          in_=iv8[:, a:b, :])
                    H8 = 8 * F
                    # Q/K are 2-series: xr8 is only consumed by the V
                    # tiles, so it loads after the K weights.
                    nc.sync.dma_start(out=w_sb["wqr8"][:, 0:H8],
                                      in_=w_d["wqr8"][:, 0:H8])
                    nc.sync.dma_start(out=w_sb["wqr8"][:, H8:2 * H8],
                                      in_=w_d["wqr8"][:, H8:2 * H8])
                    for nm in ("wk8", "wkr8"):
                        nc.sync.dma_start(out=w_sb[nm][:, 0:H8],
                                          in_=w_d[nm][:, 0:H8])
                        nc.sync.dma_start(out=w_sb[nm][:, H8:2 * H8],
                                          in_=w_d[nm][:, H8:2 * H8])
                    nc.sync.dma_start(out=tr8v[:, 0:8, :],
                                      in_=ivr[:, 0:8, :])
                    nc.sync.dma_start(out=tr8v[:, 8:N_DT, :],
                                      in_=ivr[:, 8:N_DT, :])
                    for nm in ("wv8", "wvr8"):
                        nc.sync.dma_start(out=w_sb[nm][:, :],
                                          in_=w_d[nm][:, :])
                    xc8[0], xcr8[0] = t8, tr8

                def late_weights():
                    # id/wo8/wor8 are first needed by tp0/p3 in window 1 --
                    # load them after x1 so they don't delay the pipeline
                    nc.sync.dma_start(out=id_sb[:, :], in_=id_d[:, :])
                    if n_mask:
                        nc.sync.dma_start(out=me_sb[:, :], in_=me_d[:, :])
                    nc.sync.dma_start(out=wo8_sb[:, :], in_=wo8_d[:, :])
                    nc.sync.dma_start(out=wor8_sb[:, :], in_=wor8_d[:, :])

                def p1_gen(c, part="all"):
                    """Q,K (feature-major + RoPE) and V (token-major) for
                    token chunk c.  A generator yielding after each
                    ~8-matmul quantum so attention emission can interleave
                    this PE-dense fill into its exp-bound stream.  part
                    selects the QK tiles, the V tiles, or both."""
                    mark(f"p1c{c}")
                    x8t, xr8t = xc8[c], xcr8[c]
                    xv8 = x8t[:, :].rearrange("p (d t) -> p d t", t=TC)
                    xvr = xr8t[:, :].rearrange("p (d t) -> p d t", t=TC)
                    # all-Q first: Q ft1-3 reuse wq8/x0 already on chip,
                    # so the PE chews them while the K weights stream in
                    # (chunk 0 is DMA-rate-bound at startup)
                    qk_order = [("wq8", "wqr8", qh, 0),
                                ("wq8", "wqr8", qh, 1),
                                ("wq8", "wqr8", qh, 2),
                                ("wq8", "wqr8", qh, 3),
                                ("wk8", "wkr8", kh, 0),
                                ("wk8", "wkr8", kh, 1),
                                ("wk8", "wkr8", kh, 2),
                                ("wk8", "wkr8", kh, 3)]
                    tab_tiles = {}
                    if part == "v":
                        qk_order = []
                    for (w8nm, wr8nm, dst, ft) in qk_order:
                        wv8 = w_sb[w8nm][:, :].rearrange(
                            "p (d f) -> p d f", f=F)
                        wvr = w_sb[wr8nm][:, :].rearrange(
                            "p (d f) -> p d f", f=F)
                        if ft in tab_tiles:
                            cs_t = tab_tiles[ft]
                        else:
                            cs_t = tabpool.tile([128, 2 * TC], FP16,
                                                tag="cst", name="cs_t")
                            tin = tab_d[:, :].rearrange(
                                "p (f two t) -> p f two t", f=N_FT,
                                two=2)[:, ft, :, c * TC:(c + 1) * TC]
                            nc.gpsimd.dma_start(
                                out=cs_t[:, :].rearrange(
                                    "p (two t) -> p two t", two=2),
                                in_=tin)
                            tab_tiles[ft] = cs_t
                        fs = slice(ft * 128, (ft + 1) * 128)
                        ps = p1ps.tile([128, TC], F32, tag="p1")
                        # Q/K run 2-series (w8.x8 + wr8.x8): the dropped
                        # x-residual term costs ~1e-2 end-to-end (gate
                        # 2e-2) and saves 1/3 of the Q/K projection PE
                        # time.  V and P3 stay 3-series -- sharp softmax
                        # rows pass V errors through un-averaged.
                        series = ((wv8, xv8), (wvr, xv8))
                        n_mm = 2 * (N_DT // 2)
                        i_mm = 0
                        for wv_, xv_ in series:
                            for p8 in range(N_DT // 2):
                                nc.tensor.matmul(
                                    ps[:, :],
                                    wv_[:, 2 * p8:2 * p8 + 2, fs],
                                    xv_[:, 2 * p8:2 * p8 + 2, :],
                                    start=(i_mm == 0),
                                    stop=(i_mm == n_mm - 1),
                                    perf_mode=DRMODE)
                                i_mm += 1
                            yield
                        # RoPE: stage (descale), rotate (DVE).  The
                        # descale reads PSUM, so it can ride ACT; chunks
                        # 0-2 do (ACT has slack there), chunk 3's land in
                        # the exp-saturated tail so they stay on DVE.
                        t_st = rtpool.tile([128, TC], FP16, tag="tst")
                        if c < N_TC - 1:
                            nc.scalar.activation(t_st[:, :], ps[:, :],
                                                 AF.Copy, scale=DESCALE)
                        else:
                            nc.vector.tensor_scalar_mul(t_st[:, :],
                                                        ps[:, :], DESCALE)
                        t1 = rtpool.tile([128, TC], FP16, tag="t1")
                        nc.vector.tensor_mul(t1[:, :], t_st[:, :],
                                             cs_t[:, 0:TC])
                        cs = slice(c * TC, (c + 1) * TC)
                        st_tab = cs_t[:, TC:2 * TC]
                        # st table rows are pre-swapped host-side so
                        # both DVE inputs share a base partition
                        # (walrus NCC_IBIR297).
                        for hb in (0, 64):
                            nc.vector.tensor_mul(
                                dst[ft][hb:hb + 32, cs],
                                t_st[hb + 32:hb + 64, :],
                                st_tab[hb + 32:hb + 64, :])
                            nc.vector.tensor_mul(
                                dst[ft][hb + 32:hb + 64, cs],
                                t_st[hb:hb + 32, :],
                                st_tab[hb:hb + 32, :])
                        nc.vector.tensor_add(dst[ft][:, cs],
                                             dst[ft][:, cs], t1[:, :])
                        yield
                    # V token-major
                    wv8 = w_sb["wv8"][:, :].rearrange("p (d f) -> p d f",
                                                      f=F)
                    wvr = w_sb["wvr8"][:, :].rearrange("p (d f) -> p d f",
                                                      f=F)
                    for tt in (range(TC // 128) if part != "qk" else ()):
                        kt = c * (TC // 128) + tt
                        tsl = slice(tt * 128, (tt + 1) * 128)
                        psv = p1ps.tile([128, F], F32, tag="p1")
                        series = ((xv8, wv8), (xv8, wvr), (xvr, wv8))
                        n_mm = 3 * (N_DT // 2)
                        i_mm = 0
                        for xv_, wv_ in series:
                            for p8 in range(N_DT // 2):
                                nc.tensor.matmul(
                                    psv[:, :],
                                    xv_[:, 2 * p8:2 * p8 + 2, tsl],
                                    wv_[:, 2 * p8:2 * p8 + 2, :],
                                    start=(i_mm == 0),
                                    stop=(i_mm == n_mm - 1),
                                    perf_mode=DRMODE)
                                i_mm += 1
                            yield
                        base = kt * H_LOC * 65
                        vout = v_sb[:, base:base + H_LOC * 65].rearrange(
                            "p (h c) -> p h c", c=65)[:, :, 0:64]
                        vin = psv[:, :].rearrange("p (h c) -> p h c", c=64)
                        # ACT can read PSUM (GPSIMD cannot); keeps DVE free
                        # for the RoPE stream.
                        nc.scalar.activation(vout, vin, AF.Copy,
                                             scale=DESCALE)
                        yield

                def att_ctx(qc, reg=None):
                    kinds = [block_kind[kt][qc] for kt in range(N_KT)]
                    live = [kt for kt in range(N_KT) if kinds[kt] != "skip"]
                    if not live:
                        return None
                    otok = otkpool.tile([128, N_TC * TC], FP16, tag="otok")
                    if reg is not None:
                        reg[qc] = otok
                    offs = {}
                    for kt in live:
                        if kinds[kt] == "tri":
                            offs[kt] = max(0, kt * 128 - qc * TC)
                        else:
                            offs[kt] = 0
                    R = TC // 128
                    lv = {qt: [kt for kt in live if offs[kt] <= qt * 128]
                          for qt in range(R)}
                    pairs = [live[i:i + 2] for i in range(0, len(live), 2)]
                    return dict(qc=qc, kinds=kinds, offs=offs, lv=lv,
                                pairs=pairs, otok=otok, R=R)

                def att_head_gen(ctx, h):
                    """One head\'s attention as a generator: step j emits
                    AV(pair j-1) + S/exp(pair j).  Two heads driven
                    alternately + PE fill quanta between steps hide the
                    exp latency."""
                    qc = ctx["qc"]
                    mark(f"a{qc}h{h}")
                    kinds, offs, lv = ctx["kinds"], ctx["offs"], ctx["lv"]
                    otok, R = ctx["otok"], ctx["R"]
                    ft_h = h // 2
                    po = (h % 2) * 64
                    qs = qh[ft_h][po:po + 64, :]
                    ks = kh[ft_h][po:po + 64, :]
                    # accumulator: even heads on aps, odd heads on yps; the
                    # last chunk borrows the dead projection ring so yps and
                    # aps stay free for the P3/tp fillers.
                    if qc == N_TC - 1:
                        psA_t = p1ps.tile([128, TC], F32, tag="p1",
                                          name="psA_t")
                        psA = psA_t[:, 0:R * 65]
                    elif h % 2 == 0:
                        psA = aps.tile([128, R * 65], F32, tag="av")
                    else:
                        psA_t = yps.tile([128, TC], F32, tag="y",
                                         name="psA_t")
                        psA = psA_t[:, 0:R * 65]
                    # start=True ONLY on the first AV matmul of the head:
                    # it zeroes the whole 2KB zero-region (all qt slots);
                    # later slots first-touch as overwrite-pending (= add
                    # onto zero), then pure-accumulate.  A start on any
                    # later matmul would wipe sibling slots.
                    first_mm = [True]

                    def S(pair):
                        ps = sps.tile([128, 2 * TC], F32, tag="s2")
                        es = espool.tile([128, 2 * TC], FP16, tag="es")
                        for i, kt in enumerate(pair):
                            off = offs[kt]
                            nc.tensor.matmul(
                                ps[:, i * TC + off:(i + 1) * TC],
                                ks[:, kt * 128:(kt + 1) * 128],
                                qs[:, qc * TC + off:(qc + 1) * TC],
                                start=True, stop=True)
                        lo = offs[pair[0]]
                        hi = len(pair) * TC
                        nc.scalar.activation(es[:, lo:hi], ps[:, lo:hi],
                                             AF.Exp, scale=0.125)
                        for i, kt in enumerate(pair):
                            off = offs[kt]
                            if kinds[kt] == "tri":
                                # causal triangle fixup on GPSIMD: zero
                                # es[p, j] where j < p (j relative to the
                                # block\'s diagonal at column off)
                                esl = es[:, i * TC + off:i * TC + off + 128]
                                nc.gpsimd.affine_select(
                                    esl, esl, pattern=[[1, 128]],
                                    compare_op=AluOpType.is_ge,
                                    fill=0.0, base=0,
                                    channel_multiplier=-1)
                            elif kinds[kt] == "mask":
                                ms = mask_idx[(kt, qc)]
                                nc.gpsimd.tensor_mul(
                                    es[:, i * TC:(i + 1) * TC],
                                    es[:, i * TC:(i + 1) * TC],
                                    me_sb[:, ms * TC:(ms + 1) * TC])
                        return es

                    def AV(pair, es):
                        # token-major AV (ones column -> softmax sums)
                        for i, kt in enumerate(pair):
                            for qt in range(R):
                                if kt not in lv[qt]:
                                    continue
                                nc.tensor.matmul(
                                    psA[:, qt * 65:qt * 65 + 65],
                                    es[:, i * TC + qt * 128:
                                       i * TC + (qt + 1) * 128],
                                    v_sb[:, kt * H_LOC * 65 + h * 65:
                                         kt * H_LOC * 65 + h * 65 + 65],
                                    start=first_mm[0],
                                    stop=(kt == lv[qt][-1]),
                                    skip_group_check=True)
                                first_mm[0] = False

                    pairs = ctx["pairs"]
                    prev = pairs[0]
                    es_prev = S(prev)
                    yield
                    for pair in pairs[1:]:
                        es_new = S(pair)
                        AV(prev, es_prev)
                        prev, es_prev = pair, es_new
                        yield
                    AV(prev, es_prev)
                    rcp = rcpool.tile([128, R], F32, tag="rcp")
                    sums_v = psA[:, :].rearrange("p (r c) -> p r c",
                                                 c=65)[:, :, 64]
                    nc.vector.reciprocal_approx_fast(rcp[:, :], sums_v)
                    in0 = psA[:, :].rearrange("p (r c) -> p r c",
                                              c=65)[:, :, 0:64]
                    in1 = rcp[:, :].unsqueeze(2).broadcast_to([128, R, 64])
                    out_v = otok[:, :].rearrange(
                        "p (r f) -> p r f", f=TC)[:, :, h * 64:(h + 1) * 64]
                    nc.vector.tensor_mul(out_v, in0, in1)

                def tp_gen(qc, otok, fks=None, qts=None):
                    # transpose O_tok -> O^T (feature-major)
                    mark(f"tp{qc}")
                    Rl = TC // 128
                    for qt in (range(Rl) if qts is None else qts):
                        for fk in (range(N_FT) if fks is None else fks):
                            pst = aps.tile([128, Rl * 65], F32, tag="av")
                            tpv = pst[:, 0:64].bitcast(FP16)
                            nc.tensor.transpose(
                                tpv,
                                otok[:, qt * TC + fk * 128:
                                     qt * TC + (fk + 1) * 128],
                                id_sb[:, :])
                            tglob = qc * Rl + qt
                            osl = slice(fk * T + tglob * 128,
                                        fk * T + (tglob + 1) * 128)
                            nc.vector.tensor_scalar_mul(
                                ot8_sb[:, osl], tpv, SO)
                            nc.vector.scalar_tensor_tensor(
                                otr8_sb[:, osl], tpv, SO,
                                ot8_sb[:, osl], AluOpType.mult,
                                AluOpType.subtract)
                        yield

                def p3_gen(tt, mode="w12", last=False):
                    """P3 y tile for token tile tt, yielding per dc so the
                    psy ring waits hide under interleaved attention steps.
                    mode picks which psum banks psy may ride:
                      w12 : yps / p1ps alternate (windows 1-2)
                      w3  : yps / aps alternate (window 3: p1ps = psA)
                      tail: yps / sps halves / p1ps rotation (post-att)
                    """
                    mark(f"p3t{tt}")
                    ysr = yspool.tile([128, DIM], FP16, tag="ysr")
                    sps_t = [None]
                    for dc in range(DIM // TC):
                        if dc % 2 == 0 and mode != "tail":
                            psy = yps.tile([128, TC], F32, tag="y")
                        elif mode == "w12":
                            psy = p1ps.tile([128, TC], F32, tag="p1",
                                            name="psyB")
                        elif mode == "w3":
                            psy = aps.tile([128, TC], F32, tag="av",
                                           name="psyA")
                        elif mode == "tail":
                            if dc == 0:
                                psy = yps.tile([128, TC], F32, tag="y")
                            elif dc == 2:
                                psy = p1ps.tile([128, TC], F32, tag="p1",
                                                name="psy3")
                            else:
                                if sps_t[0] is None:
                                    sps_t[0] = sps.tile([128, 2 * TC], F32,
                                                        tag="s2",
                                                        name="psyS")
                                psy = sps_t[0][:, (dc // 2) * TC:
                                               (dc // 2) * TC + TC]
                        i_mm = 0
                        for (lt, rt) in ((ot8_sb, wo8_sb),
                                         (otr8_sb, wo8_sb),
                                         (ot8_sb, wor8_sb)):
                            ltv = lt[:, :].rearrange("p (f t) -> p f t",
                                                     t=T)
                            rtv = rt[:, :].rearrange("p (f d) -> p f d",
                                                     d=DIM)
                            for fkp in range(N_FT // 2):
                                nc.tensor.matmul(
                                    psy[:, :],
                                    ltv[:, 2 * fkp:2 * fkp + 2,
                                        tt * 128:(tt + 1) * 128],
                                    rtv[:, 2 * fkp:2 * fkp + 2,
                                        dc * TC:(dc + 1) * TC],
                                    start=(i_mm == 0), stop=(i_mm == 5),
                                    perf_mode=DRMODE)
                                i_mm += 1
                        if dc % 2 == 1 and mode != "w3" and not last:
                            nc.scalar.activation(
                                ysr[:, dc * TC:(dc + 1) * TC],
                                psy[:, :], AF.Copy, scale=P3DESC)
                        else:
                            nc.vector.tensor_scalar_mul(
                                ysr[:, dc * TC:(dc + 1) * TC], psy[:, :],
                                P3DESC)
                        if mode == "tail" and last:
                            # last tile: store per dc so the final DMA
                            # chain starts as early as possible
                            nc.sync.dma_start(
                                out=y_d[tt * 128:(tt + 1) * 128,
                                        dc * TC:(dc + 1) * TC],
                                in_=ysr[:, dc * TC:(dc + 1) * TC])
                        elif mode == "tail" and dc % 2 == 1:
                            nc.sync.dma_start(
                                out=y_d[tt * 128:(tt + 1) * 128,
                                        (dc - 1) * TC:(dc + 1) * TC],
                                in_=ysr[:, (dc - 1) * TC:(dc + 1) * TC])
                        yield
                    if mode != "tail":
                        nc.sync.dma_start(
                            out=y_d[tt * 128:(tt + 1) * 128, :],
                            in_=ysr[:, :])

                def drive(pair_gens, mid=(), per_step=1.0, boundary=None,
                          drain_at=None):
                    """Drive head-pair generator groups; advance ~per_step
                    mid-fill quanta per attention step; drain boundary
                    generators after each head pair.  drain_at forces the
                    mid fill to finish before that pair index (used when a
                    slid-in next-chunk pair consumes the fill's output)."""
                    mid = list(mid)
                    quota = [0.0]

                    def adv(n):
                        quota[0] += n
                        while quota[0] >= 1.0 and mid:
                            try:
                                next(mid[0])
                            except StopIteration:
                                mid.pop(0)
                                continue
                            quota[0] -= 1.0

                    for pi, gens in enumerate(pair_gens):
                        gens = list(gens)
                        while gens:
                            for g_ in list(gens):
                                try:
                                    next(g_)
                                except StopIteration:
                                    gens.remove(g_)
                                    continue
                                adv(ps_i)
                        if boundary and pi in boundary:
                            for g_ in boundary[pi]:
                                for _ in g_:
                                    pass
                    for g_ in mid:
                        for _ in g_:
                            pass

                def hp(ctx):
                    return [[att_head_gen(ctx, h), att_head_gen(ctx, h + 1)]
                            for h in range(0, H_LOC, 2)]

                # ---- interleaved emission (software pipeline) ----
                R = TC // 128
                st_ot = {}
                startup()
                load_x(1)
                late_weights()
                for _ in p1_gen(0):
                    pass
                load_x(2)
                # window 0: att0 (tiny) carries p1c1 as fill
                drive(hp(att_ctx(0, reg=st_ot)), mid=[p1_gen(1)],
                      per_step=3.0)
                load_x(3)
                # window 1: att1 + p1c2 fill; tp0/p3(0..3) at boundaries
                drive(hp(att_ctx(1, reg=st_ot)), mid=[p1_gen(2)],
                      per_step=1.35,
                      boundary={0: [tp_gen(0, st_ot[0])],
                                1: [p3_gen(0, mode="w3")],
                                2: [p3_gen(1, mode="w3")],
                                3: [p3_gen(2, mode="w3"),
                                    p3_gen(3, mode="w3")]})
                # window 2: att2 + p1c3 fill, then att3's first head pair
                # as a 5th top-level pair (its psA rides the p1ps ring,
                # free once the p1c3 mid-fill has drained); p3(4..5) at
                # boundaries on yps/aps ("w3" mode).
                ctx3 = att_ctx(3, reg=st_ot)
                drive([[att_head_gen(ctx2, h), att_head_gen(ctx2, h + 1)]
                       for h in (2, 4, 6)] +
                      [[att_head_gen(ctx3, 0), att_head_gen(ctx3, 1)]],
                      mid=[p1_gen(3)],
                      per_step=1.0,
                      boundary={0: [tp_gen(1, st_ot[1])],
                                1: [p3_gen(4, mode="w3")],
                                2: [p3_gen(5, mode="w3")]})  # all-DVE copies
                # window 3: att3 heads 2-7; tp3.fk0 valid already (heads
                # 0-1 done), further fk-slices as pairs complete
                drive([[att_head_gen(ctx3, h), att_head_gen(ctx3, h + 1)]
                       for h in (2, 4, 6)],
                      mid=[p3_gen(6, mode="w3"),
                           tp_gen(2, st_ot[2]),
                           tp_gen(3, st_ot[3], fks=[0])] +
                          [p3_gen(t, mode="w3") for t in range(7, 12)],
                      per_step=0.65,
                      boundary={0: [tp_gen(3, st_ot[3], fks=[1])],
                                1: [tp_gen(3, st_ot[3], fks=[2])]})
                for _ in tp_gen(3, st_ot[3], fks=[3]):
                    pass
                for t in range(3 * R, 4 * R):
                    for _ in p3_gen(t, mode="tail", last=(t == 4 * R - 1)):
                        pass

                if dbg:
                    for ft in range(N_FT):
                        nc.sync.dma_start(
                            out=qh_dump[:, ft * T:(ft + 1) * T],
                            in_=qh[ft][:, :])
                        nc.sync.dma_start(
                            out=kh_dump[:, ft * T:(ft + 1) * T],
                            in_=kh[ft][:, :])
                        nc.sync.dma_start(
                            out=ot_dump[:, ft * T:(ft + 1) * T],
                            in_=ot8_sb[:, ft * T:(ft + 1) * T])
                    nc.sync.dma_start(out=v_dump[:, :], in_=v_sb[:, :])

    nc.compile()
    return nc


# ---------------- host-side preparation ----------------

def _perm():
    p = np.empty(HD, np.int64)
    p[:32] = np.arange(0, HD, 2)
    p[32:] = np.arange(1, HD, 2)
    return p


def _q8_pair(a, scale):
    """Same-grid fp8 main + residual (both at `scale`)."""
    v = np.asarray(a, np.float32) * scale
    a8 = v.astype(NPF8)
    r = v - a8.astype(np.float32)
    r8 = r.astype(NPF8)
    return a8, r8


def _pack_dmajor(mat, inner):
    """[D, inner] -> [128, N_DT*inner], col = d*inner + j."""
    d = mat.shape[0]
    return np.ascontiguousarray(
        mat.reshape(d // 128, 128, inner).transpose(1, 0, 2).reshape(
            128, (d // 128) * inner))


def _rope_tab(start_pos, heads):
    """[128, N_FT*2*T] fp16: per ft [ct | st]; rows per head block:
    ct = cos(theta_g), st = [-sin; +sin] in 32-row halves."""
    base = 1.0 / (10000.0 ** (np.arange(0, HD, 2, dtype=np.float64) / HD))
    pos = start_pos + np.arange(T, dtype=np.float64)
    tab = np.empty((128, N_FT * 2 * T), np.float32)
    for ft in range(N_FT):
        for hb in range(2):
            g = heads[2 * ft + hb]
            th = base[g] * pos
            c = np.cos(th).astype(np.float32)
            s = np.sin(th).astype(np.float32)
            r0 = hb * 64
            tab[r0:r0 + 64, ft * 2 * T:ft * 2 * T + T] = c[None, :]
            # st rows pre-swapped: top half multiplies the odd block
            # (writes even rows with -sin read from odd partitions), so
            # +sin sits at rows 0:32 and -sin at rows 32:64.
            tab[r0:r0 + 32, ft * 2 * T + T:ft * 2 * T + 2 * T] = s[None, :]
            tab[r0 + 32:r0 + 64,
                ft * 2 * T + T:ft * 2 * T + 2 * T] = -s[None, :]
    return tab.astype(np.float16)


def _classify_mask(mask):
    n_kt, n_tc = N_KT, N_TC
    tri = (np.arange(TC)[None, :] >= np.arange(128)[:, None])
    kinds = [[None] * n_tc for _ in range(n_kt)]
    for kt in range(n_kt):
        for qc in range(n_tc):
            blk = mask[qc * TC:(qc + 1) * TC, kt * 128:(kt + 1) * 128].T
            if np.all(blk <= -1e8):
                kinds[kt][qc] = "skip"
            elif np.all(blk == 0.0):
                kinds[kt][qc] = "full"
            else:
                off = kt * 128 - qc * TC
                is_tri = False
                if 0 <= off <= TC - 128:
                    ref = np.full((128, TC), -1e9, np.float32)
                    ref[:, off:] = np.where(tri[:, :TC - off], 0.0, -1e9)
                    is_tri = bool(np.array_equal(blk, ref))
                kinds[kt][qc] = "tri" if is_tri else "mask"
    slabs = []
    for kt in range(n_kt):
        for qc in range(n_tc):
            if kinds[kt][qc] == "mask":
                blk = mask[qc * TC:(qc + 1) * TC,
                           kt * 128:(kt + 1) * 128].T
                slabs.append(np.exp(blk.astype(np.float64)
                                    ).astype(np.float16))
    me = (np.concatenate(slabs, axis=1) if slabs
          else np.zeros((128, TC), np.float16))
    return kinds, me


def prepare_in_maps(x, wq, wk, wv, wo, mask, start_pos):
    x = np.asarray(x, np.float32)
    wq = np.asarray(wq, np.float32)
    wk = np.asarray(wk, np.float32)
    wv = np.asarray(wv, np.float32)
    wo = np.asarray(wo, np.float32)
    mask2d = np.asarray(mask, np.float32).reshape(mask.shape[-2],
                                                  mask.shape[-1])
    sp = int(np.asarray(start_pos))
    kinds, me = _classify_mask(mask2d)

    ident = np.eye(128, dtype=np.float16)
    perm = _perm()

    in_maps = []
    for core in range(N_CORES):
        b = core // TP
        tp = core % TP
        heads = np.arange(tp * H_LOC, (tp + 1) * H_LOC)
        rows = (heads[:, None] * HD + perm[None, :]).reshape(-1)
        rows_plain = (heads[:, None] * HD
                      + np.arange(HD)[None, :]).reshape(-1)

        xT = np.ascontiguousarray(x[b].T)               # [D, T]
        x8, xr8 = _q8_pair(xT, SX)
        x8 = _pack_dmajor(x8, T)
        xr8 = _pack_dmajor(xr8, T)

        def wpack(w, r):
            wT = np.ascontiguousarray(w[r, :].T)        # [D, F]
            a8, r8 = _q8_pair(wT, SW)
            return _pack_dmajor(a8, F), _pack_dmajor(r8, F)

        wq8, wqr8 = wpack(wq, rows)
        wk8, wkr8 = wpack(wk, rows)
        wv8, wvr8 = wpack(wv, rows_plain)
        wo_sT = np.ascontiguousarray(wo[:, rows_plain].T)  # [F, D]
        wo8, wor8 = _q8_pair(wo_sT, SW)
        wo8 = _pack_dmajor(wo8, DIM)
        wor8 = _pack_dmajor(wor8, DIM)

        in_maps.append({
            "x8": x8, "xr8": xr8,
            "wq8": wq8, "wqr8": wqr8, "wk8": wk8, "wkr8": wkr8,
            "wv8": wv8, "wvr8": wvr8,
            "tab": _rope_tab(sp, heads),
            "wo8": wo8, "wor8": wor8,
            "ident": ident, "maskexp": me,
        })
    return in_maps, kinds


_CACHE = {}


def get_nc(kinds):
    key = tuple(tuple(r) for r in kinds)
    if key not in _CACHE:
        _CACHE[key] = build_nc(kinds)
    return _CACHE[key]


def kernel(x, wq, wk, wv, wo, mask, start_pos):
    in_maps, kinds = prepare_in_maps(x, wq, wk, wv, wo, mask, start_pos)
    nc = get_nc(kinds)
    out = run_bass_kernel_spmd(nc, in_maps, core_ids=list(range(N_CORES)))
    y = np.zeros((BATCH, SEQ, DIM), np.float32)
    for core in range(N_CORES):
        y[core // TP] += out.results[core]["y"].astype(np.float32)
    return y

